# revision 2
# baseline (speedup 1.0000x reference)
"""MoE kernel v4: 8-way F-split, all experts resident on every core.

Every core holds a distinct F/8 = 512-column slice of ALL 8 experts'
w1/w2 and processes ALL routed token columns (16384 = T*top_k) on that
slice; the 8 partial outputs are summed on host, then combined/scattered
with the router weights. Per-core PE work is exactly 16384 columns x 64
cycles regardless of the routing distribution - zero load imbalance.

Token tiles are grouped per expert with balanced widths (count_e split
into ceil(count_e/512) near-equal tiles, so every matmul free dim stays
large enough to hide LDWEIGHTS). x tiles are shared by all cores; the
expert id per tile is static in the program.

DRAM layouts per core (FL = F/8 = 512, FLO = FL/128 = 4):
  x   [n_tiles, 128, KO, CT] bf16  x[t,p,ko,c] = xf[token_c, ko*128+p]
  w1  [E, 128, KO, FL]       bf16  w1[e,p,ko,f] = w1_e[ko*128+p, h*FL+f]
  w2  [E, 128, FLO, D]       bf16  w2[e,p,fo,d] = w2_e[h*FL+fo*128+p, d]
  b1  [128, E*FLO]           f32   b1[p, e*FLO+fo] = b1_e[h*FL+fo*128+p]
  y   [n_tiles, 128, KO, CT] bf16  partial (gelu(x@w1l+b1l) @ w2l)^T
(h = the core's F-slice index, 0..7.)

DMA: x reads + y writes go on the SP (sync) HWDGE queue; the 16 MB
weight stream goes on the Activation (scalar) HWDGE queue so neither
starves the other. The first tile's x and first expert's w1 are issued
in per-ko 128 KB chunks so the first matmul starts ~1.5 us in.
"""

import numpy as np
import ml_dtypes

N_CORES = 8
D = 1024
F = 4096
E = 8
KO = D // 128
FL = F // N_CORES    # 512 local F columns per core
FLO = FL // 128      # 4 local f-chunks
CT = 512

BF16 = ml_dtypes.bfloat16

_NC_CACHE: dict[tuple, object] = {}
LAST_RESULTS = None


def _balanced_tiles(C):
    """Split C columns into ceil(C/CT) near-equal tiles: [(off, w), ...]."""
    if C <= 0:
        return []
    n = (C + CT - 1) // CT
    base, rem = divmod(C, n)
    widths = [base + 1] * rem + [base] * (n - rem)
    tiles, off = [], 0
    for w in widths:
        tiles.append((off, w))
        off += w
    return tiles


def _build(spec):
    import concourse.mybir as mybir
    from concourse import bacc
    from concourse.tile import TileContext

    fp32 = mybir.dt.float32
    bf16 = mybir.dt.bfloat16

    n_tiles = len(spec)
    e_first = spec[0][0]

    nc = bacc.Bacc(
        "TRN2", target_bir_lowering=False, debug=False, num_devices=N_CORES
    )
    x = nc.dram_tensor("x", [n_tiles, 128, KO, CT], bf16, kind="ExternalInput")
    w1 = nc.dram_tensor("w1", [E, 128, KO, FL], bf16, kind="ExternalInput")
    w2 = nc.dram_tensor("w2", [E, 128, FLO, D], bf16, kind="ExternalInput")
    b1 = nc.dram_tensor("b1", [128, E * FLO], fp32, kind="ExternalInput")
    y = nc.dram_tensor("y", [n_tiles, 128, KO, CT], bf16, kind="ExternalOutput")

    with TileContext(nc) as tc:
        with (
            tc.tile_pool(name="wpool", bufs=1) as wpool,
            tc.tile_pool(name="xpool", bufs=4) as xpool,
            tc.tile_pool(name="hpool", bufs=2) as hpool,
            tc.tile_pool(name="ypool", bufs=4) as ypool,
            tc.tile_pool(name="ph", bufs=4, space="PSUM") as phpool,
            tc.tile_pool(name="py", bufs=4, space="PSUM") as pypool,
        ):
            w1_sb = wpool.tile([128, E, KO, FL], bf16)
            w2_sb = wpool.tile([128, E, FLO, D], bf16)
            b1_sb = wpool.tile([128, E * FLO], fp32)

            # First x tile and first expert's w1 in per-ko 128 KB chunks
            # (separate queues) so mm1 starts as soon as possible.
            x_first = xpool.tile([128, KO, CT], bf16, tag="x_sb")
            for ko in range(KO):
                nc.sync.dma_start(x_first[:, ko], x[0][:, ko])
            for ko in range(KO):
                nc.scalar.dma_start(w1_sb[:, e_first, ko], w1[e_first][:, ko])
            nc.scalar.dma_start(b1_sb[:], b1[:])
            nc.scalar.dma_start(w2_sb[:, e_first], w2[e_first])
            for e in range(E):
                if e == e_first:
                    continue
                nc.scalar.dma_start(w1_sb[:, e], w1[e])
                nc.scalar.dma_start(w2_sb[:, e], w2[e])

            for ti, (e, off, tw) in enumerate(spec):
                if ti == 0:
                    x_sb = x_first
                else:
                    x_sb = xpool.tile([128, KO, CT], bf16, tag="x_sb")
                    nc.sync.dma_start(x_sb[:], x[ti])
                h_sb = hpool.tile([128, FLO, CT], bf16)
                for fo in range(FLO):
                    ph = phpool.tile([128, CT], fp32)
                    for ko in range(KO):
                        nc.tensor.matmul(
                            ph[:, :tw],
                            lhsT=w1_sb[:, e, ko, fo * 128 : (fo + 1) * 128],
                            rhs=x_sb[:, ko, :tw],
                            start=(ko == 0),
                            stop=(ko == KO - 1),
                        )
                    nc.scalar.activation(
                        h_sb[:, fo, :tw],
                        ph[:, :tw],
                        mybir.ActivationFunctionType.Gelu,
                        bias=b1_sb[:, e * FLO + fo : e * FLO + fo + 1],
                    )
                for do in range(KO):
                    py = pypool.tile([128, CT], fp32)
                    for fo in range(FLO):
                        nc.tensor.matmul(
                            py[:, :tw],
                            lhsT=w2_sb[:, e, fo, do * 128 : (do + 1) * 128],
                            rhs=h_sb[:, fo, :tw],
                            start=(fo == 0),
                            stop=(fo == FLO - 1),
                        )
                    y_do = ypool.tile([128, CT], bf16, tag="y_do")
                    nc.vector.tensor_copy(y_do[:, :tw], py[:, :tw])
                    # Full-width DMA: contiguous rows; pad columns carry
                    # ignored stale data.
                    nc.sync.dma_start(y[ti][:, do, :], y_do[:])

    nc.compile()
    return nc


def kernel(x, gate_w, w1, b1, w2, b2):
    from concourse.bass_utils import run_bass_kernel_spmd

    global LAST_RESULTS

    x = np.asarray(x, dtype=np.float32)
    gate_w = np.asarray(gate_w, dtype=np.float32)
    w1 = np.asarray(w1, dtype=np.float32)
    b1 = np.asarray(b1, dtype=np.float32)
    w2 = np.asarray(w2, dtype=np.float32)
    b2 = np.asarray(b2, dtype=np.float32)

    B, S, Din = x.shape
    assert Din == D and gate_w.shape == (D, E)
    T = B * S
    xf = x.reshape(T, D)

    # ---- Host router + dispatch ----
    logits = xf.astype(np.float64) @ gate_w.astype(np.float64)
    idx0 = np.argmax(logits, axis=1)
    rows = np.arange(T)
    v0 = logits[rows, idx0]
    l2 = logits.copy()
    l2[rows, idx0] = -np.inf
    idx1 = np.argmax(l2, axis=1)
    v1_ = l2[rows, idx1]
    e1 = np.exp(v1_ - v0)
    cw0 = 1.0 / (1.0 + e1)
    cw1 = e1 / (1.0 + e1)

    token_ids = []
    combine_w = []
    for e in range(E):
        sel0 = idx0 == e
        sel1 = idx1 == e
        ids = np.nonzero(sel0 | sel1)[0]
        w = np.where(sel0[ids], cw0[ids], cw1[ids])
        token_ids.append(ids)
        combine_w.append(w)

    spec = []
    for e in range(E):
        for off, tw in _balanced_tiles(len(token_ids[e])):
            spec.append((e, off, tw))
    spec = tuple(spec)
    n_tiles = len(spec)

    if spec not in _NC_CACHE:
        _NC_CACHE[spec] = _build(spec)
    nc = _NC_CACHE[spec]

    # ---- Shared x tiles; per-core weight slices ----
    xtiles = np.zeros((n_tiles, 128, KO, CT), dtype=BF16)
    for ti, (e, off, tw) in enumerate(spec):
        ids_seg = token_ids[e][off : off + tw]
        blk = xf[ids_seg].astype(BF16).reshape(tw, KO, 128).transpose(2, 1, 0)
        xtiles[ti, :, :, :tw] = blk
    xtiles = np.ascontiguousarray(xtiles)

    b1f = b1.astype(np.float32)
    in_maps = []
    for h in range(N_CORES):
        sl = slice(h * FL, (h + 1) * FL)
        w1c = np.stack(
            [w1[e][:, sl].reshape(KO, 128, FL).transpose(1, 0, 2) for e in range(E)]
        ).astype(BF16)  # [E, 128, KO, FL]
        w2c = np.stack(
            [w2[e][sl, :].reshape(FLO, 128, D).transpose(1, 0, 2) for e in range(E)]
        ).astype(BF16)  # [E, 128, FLO, D]
        b1c = np.stack(
            [b1f[e][sl].reshape(FLO, 128).T for e in range(E)], axis=1
        ).reshape(128, E * FLO)  # [128, E*FLO]
        in_maps.append(
            {
                "x": xtiles,
                "w1": np.ascontiguousarray(w1c),
                "w2": np.ascontiguousarray(w2c),
                "b1": np.ascontiguousarray(b1c),
            }
        )

    res = run_bass_kernel_spmd(nc, in_maps, core_ids=list(range(N_CORES)))
    LAST_RESULTS = res

    # ---- Host: sum the 8 F-slice partials, combine, scatter ----
    ysum = res.results[0]["y"].astype(np.float32)
    for h in range(1, N_CORES):
        ysum += res.results[h]["y"].astype(np.float32)

    out = np.zeros((T, D), dtype=np.float32)
    for ti, (e, off, tw) in enumerate(spec):
        ids_seg = token_ids[e][off : off + tw]
        cw_seg = combine_w[e][off : off + tw].astype(np.float32)
        yt = ysum[ti, :, :, :tw].transpose(2, 1, 0).reshape(tw, D)
        out[ids_seg] += cw_seg[:, None] * (yt + b2[e])

    return out.reshape(B, S, D)
